# Initial kernel scaffold
#
"""Trainium2 Bass kernel: out = clip(x + noise, -3, 3), elementwise f32.

Full input shape (4096, 8192) f32; data-parallel over 8 NeuronCores by
slicing 512 rows per core (contiguous row blocks, no communication).
"""

import numpy as np

import concourse.bacc as bacc
import concourse.tile as tile
from concourse import mybir
from concourse.bass_utils import run_bass_kernel_spmd

N_CORES = 8
ROWS, COLS = 4096, 8192
SHARD_ROWS = ROWS // N_CORES  # 512
MIN_VAL, MAX_VAL = -3.0, 3.0

P = 128          # SBUF partitions
TW = 4096        # tile free-dim width: [128, 4096] f32 = 2 MiB per DMA

# Knobs for test.py (harness just calls kernel()).
TRACE = False
TRACE_KWARGS = {}
LAST = None  # BassKernelResults of the most recent kernel() call

_nc_cache = None


def _build():
    nc = bacc.Bacc(
        "TRN2",
        target_bir_lowering=False,
        debug=False,
        enable_asserts=False,
        num_devices=N_CORES,
    )
    x_ap = nc.dram_tensor(
        "x", [SHARD_ROWS, COLS], mybir.dt.float32, kind="ExternalInput"
    ).ap()
    n_ap = nc.dram_tensor(
        "noise", [SHARD_ROWS, COLS], mybir.dt.float32, kind="ExternalInput"
    ).ap()
    o_ap = nc.dram_tensor(
        "out", [SHARD_ROWS, COLS], mybir.dt.float32, kind="ExternalOutput"
    ).ap()

    n_row = SHARD_ROWS // P
    n_col = COLS // TW

    with tile.TileContext(nc) as tc:
        with (
            tc.tile_pool(name="xp", bufs=3) as xp,
            tc.tile_pool(name="np", bufs=3) as npool,
        ):
            for r in range(n_row):
                for c in range(n_col):
                    rs = slice(r * P, (r + 1) * P)
                    cs = slice(c * TW, (c + 1) * TW)
                    xt = xp.tile([P, TW], mybir.dt.float32)
                    nc.sync.dma_start(out=xt[:], in_=x_ap[rs, cs])
                    nt = npool.tile([P, TW], mybir.dt.float32)
                    nc.sync.dma_start(out=nt[:], in_=n_ap[rs, cs])
                    nc.vector.tensor_tensor(
                        xt[:], xt[:], nt[:], mybir.AluOpType.add
                    )
                    nc.vector.tensor_scalar(
                        xt[:],
                        xt[:],
                        MIN_VAL,
                        MAX_VAL,
                        mybir.AluOpType.max,
                        mybir.AluOpType.min,
                    )
                    nc.scalar.dma_start(out=o_ap[rs, cs], in_=xt[:])
    nc.compile()
    return nc


def kernel(x: np.ndarray, noise: np.ndarray) -> np.ndarray:
    global _nc_cache, LAST
    if _nc_cache is None:
        _nc_cache = _build()
    nc = _nc_cache

    x = np.asarray(x, dtype=np.float32)
    noise = np.asarray(noise, dtype=np.float32)
    in_maps = [
        {
            "x": x[i * SHARD_ROWS : (i + 1) * SHARD_ROWS],
            "noise": noise[i * SHARD_ROWS : (i + 1) * SHARD_ROWS],
        }
        for i in range(N_CORES)
    ]
    LAST = run_bass_kernel_spmd(
        nc, in_maps, list(range(N_CORES)), trace=TRACE, **TRACE_KWARGS
    )
    return np.concatenate([r["out"] for r in LAST.results], axis=0)


# revision 3
# speedup vs baseline: 1.2614x; 1.2614x over previous
"""Trainium2 Bass kernel: out = clip(x + noise, -3, 3), elementwise f32.

Full input shape (4096, 8192) f32; data-parallel over 8 NeuronCores by
slicing 512 rows per core (contiguous row blocks, no communication).
"""

import numpy as np

import concourse.bacc as bacc
import concourse.tile as tile
from concourse import mybir
from concourse.bass_utils import run_bass_kernel_spmd

N_CORES = 8
ROWS, COLS = 4096, 8192
SHARD_ROWS = ROWS // N_CORES  # 512
MIN_VAL, MAX_VAL = -3.0, 3.0

P = 128          # SBUF partitions
TW = 4096        # tile free-dim width: [128, 4096] f32 = 2 MiB per DMA

# Knobs for test.py (harness just calls kernel()).
TRACE = False
TRACE_KWARGS = {}
LAST = None  # BassKernelResults of the most recent kernel() call

_nc_cache = None


def _build(repeat: int = 1):
    nc = bacc.Bacc(
        "TRN2",
        target_bir_lowering=False,
        debug=False,
        enable_asserts=False,
        num_devices=N_CORES,
    )
    x_ap = nc.dram_tensor(
        "x", [SHARD_ROWS, COLS], mybir.dt.float32, kind="ExternalInput"
    ).ap()
    n_ap = nc.dram_tensor(
        "noise", [SHARD_ROWS, COLS], mybir.dt.float32, kind="ExternalInput"
    ).ap()
    o_ap = nc.dram_tensor(
        "out", [SHARD_ROWS, COLS], mybir.dt.float32, kind="ExternalOutput"
    ).ap()

    n_row = SHARD_ROWS // P
    n_col = COLS // TW

    with tile.TileContext(nc) as tc:
        with (
            tc.tile_pool(name="xp", bufs=3) as xp,
            tc.tile_pool(name="np", bufs=3) as npool,
        ):
            for r in range(n_row * repeat):
                r = r % n_row
                for c in range(n_col):
                    rs = slice(r * P, (r + 1) * P)
                    cs = slice(c * TW, (c + 1) * TW)
                    xt = xp.tile([P, TW], mybir.dt.float32)
                    nc.sync.dma_start(out=xt[:], in_=x_ap[rs, cs])
                    nt = npool.tile([P, TW], mybir.dt.float32)
                    nc.sync.dma_start(out=nt[:], in_=n_ap[rs, cs])
                    nc.vector.tensor_tensor(
                        xt[:], xt[:], nt[:], mybir.AluOpType.add
                    )
                    nc.vector.tensor_scalar(
                        xt[:],
                        xt[:],
                        MIN_VAL,
                        MAX_VAL,
                        mybir.AluOpType.max,
                        mybir.AluOpType.min,
                    )
                    nc.scalar.dma_start(out=o_ap[rs, cs], in_=xt[:])
    nc.compile()
    return nc


def kernel(x: np.ndarray, noise: np.ndarray) -> np.ndarray:
    global _nc_cache, LAST
    if _nc_cache is None:
        _nc_cache = _build()
    nc = _nc_cache

    x = np.asarray(x, dtype=np.float32)
    noise = np.asarray(noise, dtype=np.float32)
    in_maps = [
        {
            "x": x[i * SHARD_ROWS : (i + 1) * SHARD_ROWS],
            "noise": noise[i * SHARD_ROWS : (i + 1) * SHARD_ROWS],
        }
        for i in range(N_CORES)
    ]
    LAST = run_bass_kernel_spmd(
        nc, in_maps, list(range(N_CORES)), trace=TRACE, **TRACE_KWARGS
    )
    return np.concatenate([r["out"] for r in LAST.results], axis=0)
